# revision 32
# baseline (speedup 1.0000x reference)
"""Trainium2 Bass kernel for nn_MFA_87067577025371.

Architecture (B=2, C=64, Ci=32, H=W=96, N=9216):
  k,v = 1x1conv(xA); q = 1x1conv(xB)
  A   = softmax(v^T q, axis=2)            # [B, N, N], softmax over query dim m
  av  = k @ A                             # [B, Ci, N]
  out = relu(BN2(Wo @ BN1(Wg @ av)) + xB)

Softmax is over the *output* (query) dim of the av contraction, so the row
statistics Z_n = sum_m exp(s[n,m]) only need rows n — we shard over
(batch, key-row chunk): each of 8 cores owns N/4 = 2304 rows of the score
matrix for one batch.

Phase-1 design (per core):
 - scores S = vv^T xB_aug with vv = M^T xA_aug, M = [[Wv^T Wq, Wv^T bq],
   [bv^T Wq, bv.bq]] host-folded: one fp8 DoubleRow matmul per 512-chunk
   (contraction 66 = 2x33 subtiles; xB_aug host-cast into the split fp8
   layout, vv computed on-core and split by two PSUM->SBUF cast copies).
 - exp is split across two engines: ACT runs the true Exp (bias=-c,
   fused row-sum accumulator); DVE runs a custom fused 8-stage op
   ((x*C0+C1)^2+0.5)^8 = (1+t+t^2/2)^8 ~= e^(x-c), t=(x-c)/8, also with
   fused accumulation.  E is stored fp8 (shift c keeps it in range; the
   softmax is shift-invariant so c cancels exactly).
 - av: adjacent row-blocks are paired; each pair is one fp8 DoubleRow
   matmul (2x128 contraction per 0.5-cycle column).  Pairs 0-3 chain in
   PSUM per 512-chunk (slots spread over later blocks' exp strips), one
   copy into a bf16 accumulator; pairs 4-8 chain in the tail and a single
   DVE add merges accumulator + tail chain into the f32 output chunk.
A tiny second kernel sums the 4 partials per batch and applies the epilogue
(all BN/conv algebra is host-folded into one 64x32 matmul + bias).
"""

import os
import sys

import numpy as np

for _p in ("/opt/trn_rl_repo", "/root/.axon_site/_ro/trn_rl_repo"):
    if os.path.isdir(_p) and _p not in sys.path:
        sys.path.insert(0, _p)

import ml_dtypes  # noqa: E402

BF16 = ml_dtypes.bfloat16
E4M3 = ml_dtypes.float8_e4m3

# ---- problem constants (hardcoded per contract) ----
B, C, CI, H, W = 2, 64, 32, 96, 96
N = H * W                  # 9216
NCORES = 8
NCHUNK = N // 4            # 2304 key rows per core
NSUB = NCHUNK // 128       # 18 blocks of 128 rows
NPAIR = NSUB // 2          # 9 DoubleRow block pairs
STRIP = 1024               # exp strip (2 PSUM banks; 3-deep pipeline)
NSTRIP = N // STRIP        # 9
AVS = 512                  # av matmul chunk
NAVS = N // AVS            # 18
CAUG = C + 1               # 65 (bias row folded in)
KSUB = 33                  # fp8 DoubleRow contraction subtile (2x33 >= 65)
EPS = 1e-5
C_SHIFT = 4.5              # global exponent shift (softmax-invariant)
SQRT2 = 1.4142135623730951

APAIRS = 4                 # pairs 0-3 chain into av_acc (spread)
# pairs 4-8 chain in the tail directly

# exp engine split: a of the 162 strips go to ACT, rest to the DVE custom op.
ACT_STRIPS = 84

_CACHE = {}


def _assign_act(idx, total=NSUB * NSTRIP, a=ACT_STRIPS):
    return ((idx + 1) * a) // total > (idx * a) // total


def _register_exp8():
    """Runtime-register the custom DVE exp-approx op (idempotent)."""
    from concourse import dve_ops
    from concourse.dve_spec import Spec, Src0, C0, C1, C2, sq, AluOp, lower
    from concourse.dve_uop import DveOpSpec

    name = "EXP8_ANT"
    for o in dve_ops.OPS:
        if o.name == name:
            return o
    import numpy as _np

    row = dve_ops._CUSTOM_DVE_ROW_BASE + len(dve_ops.OPS)
    dve_ops._SUB_OPCODE_FOR_NAME[name] = row
    body = sq(sq(sq(sq(Src0 * C0 + C1) + C2)))
    spec = Spec(
        body=body,
        accum=AluOp.ADD,
        reference=lambda in0, in1, s0, s1, imm2: _np.float32(
            ((((in0 * s0 + s1) ** 2 + imm2) ** 2) ** 2) ** 2
        ),
    )
    shas = {
        v: DveOpSpec(name=name, opcode=row, uops=lower(spec, ver=v),
                     rd1_en=False).sha(v)
        for v in ("v3", "v4")
    }
    op = dve_ops.DveOp(name, spec, subdim=False, uops_sha=shas)
    dve_ops.OPS.append(op)
    return op


def _build_phase1():
    import concourse.bacc as bacc
    import concourse.tile as tile
    from concourse import mybir

    f32 = mybir.dt.float32
    bf16 = mybir.dt.bfloat16
    fp8 = mybir.dt.float8e4
    AX = mybir.AxisListType
    AF = mybir.ActivationFunctionType
    ADD = mybir.AluOpType.add
    DR = mybir.MatmulPerfMode.DoubleRow
    exp8 = _register_exp8()
    # DVE approx constants: ((x*C0g + C1g)^2 + 0.5)^8 ~= e^(x - C_SHIFT)
    C0G = 1.0 / (8.0 * SQRT2)
    C1G = (1.0 - C_SHIFT / 8.0) / SQRT2

    nc = bacc.Bacc("TRN2", target_bir_lowering=False, debug=False,
                   dynamic_dma_scratch_size=4096)

    xb_d = nc.dram_tensor("xb_split", [KSUB, 2 * N], fp8, kind="ExternalInput").ap()
    xa_d = nc.dram_tensor("xa_aug", [CAUG, NCHUNK], bf16, kind="ExternalInput").ap()
    mt_d = nc.dram_tensor("mt", [CAUG, 2 * KSUB], bf16, kind="ExternalInput").ap()
    wk_d = nc.dram_tensor("wk", [CAUG, CI], bf16, kind="ExternalInput").ap()
    av1_d = nc.dram_tensor("av1", [CI, N], f32, kind="ExternalOutput").ap()
    av2_d = nc.dram_tensor("av2", [CI, N], f32, kind="ExternalOutput").ap()

    with tile.TileContext(nc) as tc:
        with (
            tc.tile_pool(name="pairs", bufs=NPAIR) as pairs,
            tc.tile_pool(name="pers", bufs=1) as pers,
            tc.tile_pool(name="early", bufs=1) as early,
            tc.tile_pool(name="small", bufs=4) as small,
            tc.tile_pool(name="stats", bufs=10) as stats,
            tc.tile_pool(name="avo", bufs=6) as avo,
            tc.tile_pool(name="scp", bufs=3, space="PSUM") as scp,
            tc.tile_pool(name="avp", bufs=2, space="PSUM") as avp,
        ):
            # ---- warmup: trigger ACT exp-table load before any data ----
            warm = small.tile([128, 1], f32, tag="warm")
            nc.vector.memset(warm[:, :], 0.0)
            warm2 = small.tile([128, 1], f32, tag="warm")
            nc.scalar.activation(warm2[:, :], warm[:, :], AF.Exp)
            bias = small.tile([128, 1], f32, tag="bias")
            nc.vector.memset(bias[:, :], -C_SHIFT)

            # ---- loads: weights + first xB chunks lead; xA on SWDGE ----
            xa_sb = early.tile([CAUG, NCHUNK], bf16, tag="xa")
            nc.gpsimd.dma_start(xa_sb[:], xa_d[:])
            mt_sb = small.tile([CAUG, 2 * KSUB], bf16, tag="mt")
            nc.sync.dma_start(mt_sb[:], mt_d[:])
            wk_sb = small.tile([CAUG, CI], bf16, tag="wk")
            nc.sync.dma_start(wk_sb[:], wk_d[:])
            xb_sb = pers.tile([KSUB, 2, N], fp8, tag="xb")
            xb3 = xb_d[:].rearrange("p (t f) -> p t f", t=2)
            for blk in range(NSTRIP):
                lo, hi = blk * STRIP, (blk + 1) * STRIP
                nc.sync.dma_start(xb_sb[:, :, lo:hi], xb3[:, :, lo:hi])

            vv_sb = pers.tile([KSUB, 2, NCHUNK], fp8, tag="vv")
            kT_sb = pers.tile([128, NSUB * CI], bf16, tag="kT")

            # ---- vv = M^T xa_aug, split into 2x33 fp8 subtiles ----
            # two stationaries land the halves at PSUM partitions 0-32/64-96
            for ch in range((NCHUNK + 511) // 512):
                lo = ch * 512
                w_ = min(512, NCHUNK - lo)
                pt = avp.tile([128, 512], f32, tag="av")
                nc.tensor.matmul(pt[0:KSUB, 0:w_], mt_sb[:, 0:KSUB],
                                 xa_sb[:, lo:lo + w_], start=True, stop=True)
                nc.tensor.matmul(pt[64:64 + KSUB, 0:w_], mt_sb[:, KSUB:2 * KSUB],
                                 xa_sb[:, lo:lo + w_], start=True, stop=True,
                                 tile_position=(0, 64))
                if ch % 2 == 0:
                    nc.scalar.copy(vv_sb[:, 0, lo:lo + w_], pt[0:KSUB, 0:w_])
                    nc.vector.tensor_copy(vv_sb[:, 1, lo:lo + w_],
                                          pt[64:64 + KSUB, 0:w_])
                else:
                    nc.vector.tensor_copy(vv_sb[:, 0, lo:lo + w_],
                                          pt[0:KSUB, 0:w_])
                    nc.scalar.copy(vv_sb[:, 1, lo:lo + w_],
                                   pt[64:64 + KSUB, 0:w_])

            # ---- kT[n, c] = (Wk@xA + bk)^T : NSUB tiles of [128, CI] ----
            for half in range(2):
                pt = avp.tile([128, 512], f32, tag="av")
                js = range(9 * half, 9 * (half + 1))
                for i, j in enumerate(js):
                    nc.tensor.matmul(
                        pt[:, i * CI:(i + 1) * CI],
                        xa_sb[:, j * 128:(j + 1) * 128],
                        wk_sb[:, :],
                        start=True, stop=True,
                    )
                nc.vector.tensor_copy(
                    kT_sb[:, half * 9 * CI:(half + 1) * 9 * CI], pt[:, 0:9 * CI]
                )

            # ---- main loop ----
            e_pairs = [None] * NPAIR
            kts_pairs = [None] * NPAIR
            av_queue = []   # pending spread chunks for the A-group chain
            copy_flip = [0]

            def emit_slot(t):
                at = avp.tile([128, AVS], f32, tag="av")
                for p in range(APAIRS):
                    nc.tensor.matmul(
                        at[0:CI, :],
                        kts_pairs[p][:, :, :],
                        e_pairs[p][:, :, t * AVS:(t + 1) * AVS],
                        start=(p == 0), stop=(p == APAIRS - 1),
                        perf_mode=DR,
                    )
                ot = avo.tile([CI, AVS], f32, tag="avo")
                copy_flip[0] ^= 1
                if copy_flip[0]:
                    nc.scalar.copy(ot[:, :], at[0:CI, :])
                else:
                    nc.vector.tensor_copy(ot[:, :], at[0:CI, :])
                nc.sync.dma_start(av1_d[:, t * AVS:(t + 1) * AVS], ot[:, :])

            sidx = 0
            for j in range(NSUB):
                p, half = divmod(j, 2)
                if half == 0:
                    e_pr = pairs.tile([128, 2, N], fp8, tag="E")
                    kts_pr = stats.tile([128, 2, CI], fp8, tag="kts")
                    e_pairs[p] = e_pr
                    kts_pairs[p] = kts_pr
                e_t = e_pairs[p]
                zp = stats.tile([128, 12], f32, tag="zp")
                for s in range(NSTRIP):
                    sc = scp.tile([128, STRIP], f32, tag="sc")
                    for t3 in range(STRIP // 512):
                        col = s * STRIP + t3 * 512
                        nc.tensor.matmul(
                            sc[:, t3 * 512:(t3 + 1) * 512],
                            vv_sb[:, :, j * 128:(j + 1) * 128],
                            xb_sb[:, :, col:col + 512],
                            start=True, stop=True,
                            perf_mode=DR,
                        )
                    dst = e_t[:, half, s * STRIP:(s + 1) * STRIP]
                    if _assign_act(sidx):
                        nc.scalar.activation(dst, sc[:, :], AF.Exp,
                                             bias=bias[:, :],
                                             accum_out=zp[:, s:s + 1])
                    else:
                        nc.vector._custom_dve(exp8, out=dst, in0=sc[:, :],
                                              s0=C0G, s1=C1G, imm2=0.5,
                                              accum_out=zp[:, s:s + 1])
                    sidx += 1
                    # pace the spread: 18 slots over the ~90 remaining strips
                    remaining = (NSUB - 1 - j) * NSTRIP + (NSTRIP - 1 - s)
                    while av_queue and len(av_queue) * 5 > remaining:
                        emit_slot(av_queue.pop(0))
                z = stats.tile([128, 1], f32, tag="z")
                nc.vector.reduce_sum(z[:, :], zp[:, 0:NSTRIP], axis=AX.X)
                rinv = stats.tile([128, 1], f32, tag="rinv")
                nc.vector.reciprocal(rinv[:, :], z[:, :])
                # x64 keeps the fp8 kts out of subnormal range; phase2's
                # Wfin is pre-divided by 64 to compensate
                nc.vector.tensor_scalar(
                    kts_pairs[p][:, half, :], kT_sb[:, j * CI:(j + 1) * CI],
                    rinv[:, :], 64.0,
                    op0=mybir.AluOpType.mult, op1=mybir.AluOpType.mult,
                )
                if j == 2 * APAIRS - 1:
                    av_queue.extend(range(NAVS))

            # ---- tail: pairs 4-8 chain per chunk, 5 chunks in flight
            # across all 5 PSUM slots; pair-outer order amortizes the
            # stationary loads ----
            for tg in range(0, NAVS, 5):
                ts_ = list(range(tg, min(NAVS, tg + 5)))
                rts = {}
                for i, t in enumerate(ts_):
                    pool, tag = (scp, "sc") if i % 2 == 0 else (avp, "av")
                    rt_tile = pool.tile([128, AVS], f32, tag=tag)
                    rts[t] = rt_tile
                for p in range(APAIRS, NPAIR):
                    for t in ts_:
                        nc.tensor.matmul(
                            rts[t][0:CI, :],
                            kts_pairs[p][:, :, :],
                            e_pairs[p][:, :, t * AVS:(t + 1) * AVS],
                            start=(p == APAIRS), stop=(p == NPAIR - 1),
                            perf_mode=DR,
                        )
                for t in ts_:
                    ot = avo.tile([CI, AVS], f32, tag="avo")
                    if t % 2 == 0:
                        nc.scalar.copy(ot[:, :], rts[t][0:CI, :])
                    else:
                        nc.vector.tensor_copy(ot[:, :], rts[t][0:CI, :])
                    nc.sync.dma_start(av2_d[:, t * AVS:(t + 1) * AVS], ot[:, :])

    nc.compile()
    return nc


def _build_phase2():
    import concourse.bacc as bacc
    import concourse.tile as tile
    from concourse import mybir

    f32 = mybir.dt.float32
    bf16 = mybir.dt.bfloat16
    AF = mybir.ActivationFunctionType
    MQ = N // 4  # 2304 output columns per core

    nc = bacc.Bacc("TRN2", target_bir_lowering=False, debug=False)

    # avs carries [WfinT+cfin | av_sum+ones] side by side; chunked DMAs so
    # the first matmul starts as soon as its slice lands
    avs_d = nc.dram_tensor("avs", [CI + 1, MQ + C], bf16, kind="ExternalInput").ap()
    xbc_d = nc.dram_tensor("xbc", [C, MQ], bf16, kind="ExternalInput").ap()
    out_d = nc.dram_tensor("outc", [C, MQ], f32, kind="ExternalOutput").ap()

    with tile.TileContext(nc) as tc:
        with (
            tc.tile_pool(name="sb", bufs=1) as sb,
            tc.tile_pool(name="tp", bufs=3) as tp,
            tc.tile_pool(name="ps", bufs=6, space="PSUM") as ps,
        ):
            warm = sb.tile([128, 1], f32, tag="warm")
            nc.vector.memset(warm[:, :], 0.0)
            warm2 = sb.tile([128, 1], f32, tag="warm2")
            nc.scalar.activation(warm2[:, :], warm[:, :], AF.Relu)

            av_aug = sb.tile([CI + 1, MQ + C], bf16, tag="avaug")
            nc.sync.dma_start(av_aug[:, 0:C + 512], avs_d[:, 0:C + 512])
            xbc_sb = sb.tile([C, MQ], bf16, tag="xbc")
            nc.gpsimd.dma_start(xbc_sb[:, 0:1024], xbc_d[:, 0:1024])
            for lo in range(C + 512, MQ + C, 1024):
                hi = min(MQ + C, lo + 1024)
                nc.sync.dma_start(av_aug[:, lo:hi], avs_d[:, lo:hi])
            nc.gpsimd.dma_start(xbc_sb[:, 1024:MQ], xbc_d[:, 1024:MQ])
            o_sb = sb.tile([C, MQ], f32, tag="o")

            nstr = (MQ + 511) // 512
            for s in range(nstr):
                sw = min(512, MQ - s * 512)
                sl = slice(s * 512, s * 512 + sw)
                op = ps.tile([128, 512], f32, tag="rp")
                nc.tensor.matmul(
                    op[0:C, 0:sw], av_aug[:, 0:C],
                    av_aug[:, C + s * 512:C + s * 512 + sw],
                    start=True, stop=True,
                )
                t_sb = tp.tile([C, 512], f32, tag="t")
                nc.vector.tensor_tensor(
                    t_sb[:, 0:sw], xbc_sb[:, sl], op[0:C, 0:sw],
                    op=mybir.AluOpType.add,
                )
                nc.scalar.activation(o_sb[:, sl], t_sb[:, 0:sw], AF.Relu)
                nc.sync.dma_start(out_d[:, sl], o_sb[:, sl])

    nc.compile()
    return nc


def _get_programs():
    if "p1" not in _CACHE:
        _CACHE["p1"] = _build_phase1()
        _CACHE["p2"] = _build_phase2()
    return _CACHE["p1"], _CACHE["p2"]


def kernel(xA, xB, Wk, bk, Wv, bv, Wq, bq, Wg,
           g1_gamma, g1_beta, g1_mean, g1_var,
           Wo, bo, g2_gamma, g2_beta, g2_mean, g2_var):
    from concourse.bass_utils import run_bass_kernel_spmd

    p1, p2 = _get_programs()

    xA = np.asarray(xA, np.float32).reshape(B, C, N)
    xB = np.asarray(xB, np.float32).reshape(B, C, N)

    # ---- host-side weight folding (tiny) ----
    s1 = np.asarray(g1_gamma) / np.sqrt(np.asarray(g1_var) + EPS)
    Wg_f = s1[:, None] * np.asarray(Wg)
    c1 = np.asarray(g1_beta) - s1 * np.asarray(g1_mean)
    s2 = np.asarray(g2_gamma) / np.sqrt(np.asarray(g2_var) + EPS)
    Wo_f = s2[:, None] * np.asarray(Wo)
    c2 = s2 * (np.asarray(bo) - np.asarray(g2_mean)) + np.asarray(g2_beta)
    Wfin = (Wo_f @ Wg_f).astype(np.float32)          # [C, CI]
    cfin = (Wo_f @ c1 + c2).astype(np.float32)       # [C]

    # scores fold: S = xA_aug^T M xB_aug,  M = [[Wv^T Wq, Wv^T bq],
    #                                          [bv^T Wq,  bv.bq ]]
    Wv_, bv_ = np.asarray(Wv), np.asarray(bv)
    Wq_, bq_ = np.asarray(Wq), np.asarray(bq)
    M = np.zeros((CAUG, CAUG), np.float32)
    M[:C, :C] = Wv_.T @ Wq_
    M[:C, C] = Wv_.T @ bq_
    M[C, :C] = bv_ @ Wq_
    M[C, C] = bv_ @ bq_
    # stationary halves for vv = M^T xA_aug: mt[:, h*33+e] = M[:, ...]
    Mp = np.zeros((CAUG, 2 * KSUB), np.float32)
    Mp[:, 0:KSUB] = M[:, 0:KSUB]
    Mp[:, KSUB:KSUB + (CAUG - KSUB)] = M[:, KSUB:CAUG]
    mt16 = Mp.astype(BF16)

    wk_aug = np.concatenate([np.asarray(Wk).T, np.asarray(bk)[None, :]], 0).astype(BF16)
    # av_part carries a x64 factor (fp8 subnormal guard) — undo it in Wfin
    wfin_aug = np.concatenate([Wfin.T / 64.0, cfin[None, :]], 0).astype(BF16)

    ones_n = np.ones((1, N), np.float32)

    # xB_aug in the fp8 split layout [33, 2, N] (ch 0-32 | ch 33-64 + pad)
    xb_split = np.zeros((B, KSUB, 2, N), np.float32)
    for b in range(B):
        xb_aug = np.concatenate([xB[b], ones_n], 0)  # [65, N]
        xb_split[b, :, 0, :] = xb_aug[0:KSUB]
        xb_split[b, 0:CAUG - KSUB, 1, :] = xb_aug[KSUB:CAUG]
    xb_split8 = xb_split.reshape(B, KSUB, 2 * N).astype(E4M3)

    # ---- phase 1: per-core (batch, key-row chunk) partial attention ----
    in_maps1 = []
    for core in range(NCORES):
        b, chunk = divmod(core, 4)
        sl = slice(chunk * NCHUNK, (chunk + 1) * NCHUNK)
        in_maps1.append({
            "xb_split": xb_split8[b],
            "xa_aug": np.concatenate([xA[b][:, sl], ones_n[:, sl]], 0).astype(BF16),
            "mt": mt16,
            "wk": wk_aug,
        })
    res1 = run_bass_kernel_spmd(p1, in_maps1, list(range(NCORES)))
    av_parts = [res1.results[i]["av1"] + res1.results[i]["av2"]
                for i in range(NCORES)]

    # ---- phase 2: per-core (batch, query chunk) epilogue ----
    # (the 4-way partial sum happens on host as part of the gather)
    MQ = N // 4
    av_sum = [sum(av_parts[b * 4 + i] for i in range(4)) for b in range(B)]
    ones_mq = np.ones((1, MQ), np.float32)
    in_maps2 = []
    for core in range(NCORES):
        b, mq = divmod(core, 4)
        msl = slice(mq * MQ, (mq + 1) * MQ)
        av_aug = np.concatenate([av_sum[b][:, msl], ones_mq], 0)
        in_maps2.append({
            "avs": np.concatenate([wfin_aug, av_aug], 1).astype(BF16),
            "xbc": np.ascontiguousarray(xB[b][:, msl]).astype(BF16),
        })
    res2 = run_bass_kernel_spmd(p2, in_maps2, list(range(NCORES)))

    out = np.zeros((B, C, N), np.float32)
    for core in range(NCORES):
        b, mq = divmod(core, 4)
        out[b][:, mq * MQ:(mq + 1) * MQ] = res2.results[core]["outc"]
    return out.reshape(B, C, H, W)


# revision 33
# speedup vs baseline: 1.0170x; 1.0170x over previous
"""Trainium2 Bass kernel for nn_MFA_87067577025371.

Architecture (B=2, C=64, Ci=32, H=W=96, N=9216):
  k,v = 1x1conv(xA); q = 1x1conv(xB)
  A   = softmax(v^T q, axis=2)            # [B, N, N], softmax over query dim m
  av  = k @ A                             # [B, Ci, N]
  out = relu(BN2(Wo @ BN1(Wg @ av)) + xB)

Softmax is over the *output* (query) dim of the av contraction, so the row
statistics Z_n = sum_m exp(s[n,m]) only need rows n — we shard over
(batch, key-row chunk): each of 8 cores owns N/4 = 2304 rows of the score
matrix for one batch.

Phase-1 design (per core):
 - scores S = vv^T xB_aug with vv = M^T xA_aug, M = [[Wv^T Wq, Wv^T bq],
   [bv^T Wq, bv.bq]] host-folded: one fp8 DoubleRow matmul per 512-chunk
   (contraction 66 = 2x33 subtiles; xB_aug host-cast into the split fp8
   layout, vv computed on-core and split by two PSUM->SBUF cast copies).
 - exp is split across two engines: ACT runs the true Exp (bias=-c,
   fused row-sum accumulator); DVE runs a custom fused 8-stage op
   ((x*C0+C1)^2+0.5)^8 = (1+t+t^2/2)^8 ~= e^(x-c), t=(x-c)/8, also with
   fused accumulation.  E is stored fp8 (shift c keeps it in range; the
   softmax is shift-invariant so c cancels exactly).
 - av: adjacent row-blocks are paired; each pair is one fp8 DoubleRow
   matmul (2x128 contraction per 0.5-cycle column).  Pairs 0-3 chain in
   PSUM per 512-chunk (slots spread over later blocks' exp strips) and
   stream out as av1; pairs 4-8 chain in the tail (5 chunks in flight
   across the 5 PSUM slots) as av2; the host sums av1+av2 during the
   gather, avoiding any on-core merge serialization.
A tiny second kernel sums the 4 partials per batch and applies the epilogue
(all BN/conv algebra is host-folded into one 64x32 matmul + bias).
"""

import os
import sys

import numpy as np

for _p in ("/opt/trn_rl_repo", "/root/.axon_site/_ro/trn_rl_repo"):
    if os.path.isdir(_p) and _p not in sys.path:
        sys.path.insert(0, _p)

import ml_dtypes  # noqa: E402

BF16 = ml_dtypes.bfloat16
E4M3 = ml_dtypes.float8_e4m3

# ---- problem constants (hardcoded per contract) ----
B, C, CI, H, W = 2, 64, 32, 96, 96
N = H * W                  # 9216
NCORES = 8
NCHUNK = N // 4            # 2304 key rows per core
NSUB = NCHUNK // 128       # 18 blocks of 128 rows
NPAIR = NSUB // 2          # 9 DoubleRow block pairs
STRIP = 1024               # exp strip (2 PSUM banks; 3-deep pipeline)
NSTRIP = N // STRIP        # 9
AVS = 512                  # av matmul chunk
NAVS = N // AVS            # 18
CAUG = C + 1               # 65 (bias row folded in)
KSUB = 33                  # fp8 DoubleRow contraction subtile (2x33 >= 65)
EPS = 1e-5
C_SHIFT = 4.5              # global exponent shift (softmax-invariant)
SQRT2 = 1.4142135623730951

APAIRS = 4                 # pairs 0-3 chain into av_acc (spread)
# pairs 4-8 chain in the tail directly

# exp engine split: a of the 162 strips go to ACT, rest to the DVE custom op.
ACT_STRIPS = 82

_CACHE = {}


def _assign_act(idx, total=NSUB * NSTRIP, a=ACT_STRIPS):
    return ((idx + 1) * a) // total > (idx * a) // total


def _register_exp8():
    """Runtime-register the custom DVE exp-approx op (idempotent)."""
    from concourse import dve_ops
    from concourse.dve_spec import Spec, Src0, C0, C1, C2, sq, AluOp, lower
    from concourse.dve_uop import DveOpSpec

    name = "EXP8_ANT"
    for o in dve_ops.OPS:
        if o.name == name:
            return o
    import numpy as _np

    row = dve_ops._CUSTOM_DVE_ROW_BASE + len(dve_ops.OPS)
    dve_ops._SUB_OPCODE_FOR_NAME[name] = row
    body = sq(sq(sq(sq(Src0 * C0 + C1) + C2)))
    spec = Spec(
        body=body,
        accum=AluOp.ADD,
        reference=lambda in0, in1, s0, s1, imm2: _np.float32(
            ((((in0 * s0 + s1) ** 2 + imm2) ** 2) ** 2) ** 2
        ),
    )
    shas = {
        v: DveOpSpec(name=name, opcode=row, uops=lower(spec, ver=v),
                     rd1_en=False).sha(v)
        for v in ("v3", "v4")
    }
    op = dve_ops.DveOp(name, spec, subdim=False, uops_sha=shas)
    dve_ops.OPS.append(op)
    return op


def _build_phase1():
    import concourse.bacc as bacc
    import concourse.tile as tile
    from concourse import mybir

    f32 = mybir.dt.float32
    bf16 = mybir.dt.bfloat16
    fp8 = mybir.dt.float8e4
    AX = mybir.AxisListType
    AF = mybir.ActivationFunctionType
    ADD = mybir.AluOpType.add
    DR = mybir.MatmulPerfMode.DoubleRow
    exp8 = _register_exp8()
    # DVE approx constants: ((x*C0g + C1g)^2 + 0.5)^8 ~= e^(x - C_SHIFT)
    C0G = 1.0 / (8.0 * SQRT2)
    C1G = (1.0 - C_SHIFT / 8.0) / SQRT2

    nc = bacc.Bacc("TRN2", target_bir_lowering=False, debug=False,
                   dynamic_dma_scratch_size=4096)

    xb_d = nc.dram_tensor("xb_split", [KSUB, 2 * N], fp8, kind="ExternalInput").ap()
    xa_d = nc.dram_tensor("xa_aug", [CAUG, NCHUNK], bf16, kind="ExternalInput").ap()
    mt_d = nc.dram_tensor("mt", [CAUG, 2 * KSUB], bf16, kind="ExternalInput").ap()
    wk_d = nc.dram_tensor("wk", [CAUG, CI], bf16, kind="ExternalInput").ap()
    av1_d = nc.dram_tensor("av1", [CI, N], f32, kind="ExternalOutput").ap()
    av2_d = nc.dram_tensor("av2", [CI, N], f32, kind="ExternalOutput").ap()

    with tile.TileContext(nc) as tc:
        with (
            tc.tile_pool(name="pairs", bufs=NPAIR) as pairs,
            tc.tile_pool(name="pers", bufs=1) as pers,
            tc.tile_pool(name="early", bufs=1) as early,
            tc.tile_pool(name="small", bufs=4) as small,
            tc.tile_pool(name="stats", bufs=10) as stats,
            tc.tile_pool(name="avo", bufs=6) as avo,
            tc.tile_pool(name="scp", bufs=3, space="PSUM") as scp,
            tc.tile_pool(name="avp", bufs=2, space="PSUM") as avp,
        ):
            # ---- warmup: trigger ACT exp-table load before any data ----
            warm = small.tile([128, 1], f32, tag="warm")
            nc.vector.memset(warm[:, :], 0.0)
            warm2 = small.tile([128, 1], f32, tag="warm")
            nc.scalar.activation(warm2[:, :], warm[:, :], AF.Exp)
            bias = small.tile([128, 1], f32, tag="bias")
            nc.vector.memset(bias[:, :], -C_SHIFT)

            # ---- loads: weights + first xB chunks lead; xA on SWDGE ----
            xa_sb = early.tile([CAUG, NCHUNK], bf16, tag="xa")
            nc.gpsimd.dma_start(xa_sb[:], xa_d[:])
            mt_sb = small.tile([CAUG, 2 * KSUB], bf16, tag="mt")
            nc.sync.dma_start(mt_sb[:], mt_d[:])
            wk_sb = small.tile([CAUG, CI], bf16, tag="wk")
            nc.sync.dma_start(wk_sb[:], wk_d[:])
            xb_sb = pers.tile([KSUB, 2, N], fp8, tag="xb")
            xb3 = xb_d[:].rearrange("p (t f) -> p t f", t=2)
            for blk in range(NSTRIP):
                lo, hi = blk * STRIP, (blk + 1) * STRIP
                nc.sync.dma_start(xb_sb[:, :, lo:hi], xb3[:, :, lo:hi])

            vv_sb = pers.tile([KSUB, 2, NCHUNK], fp8, tag="vv")
            kT_sb = pers.tile([128, NSUB * CI], bf16, tag="kT")

            # ---- vv = M^T xa_aug, split into 2x33 fp8 subtiles ----
            # two stationaries land the halves at PSUM partitions 0-32/64-96
            for ch in range((NCHUNK + 511) // 512):
                lo = ch * 512
                w_ = min(512, NCHUNK - lo)
                pt = avp.tile([128, 512], f32, tag="av")
                nc.tensor.matmul(pt[0:KSUB, 0:w_], mt_sb[:, 0:KSUB],
                                 xa_sb[:, lo:lo + w_], start=True, stop=True)
                nc.tensor.matmul(pt[64:64 + KSUB, 0:w_], mt_sb[:, KSUB:2 * KSUB],
                                 xa_sb[:, lo:lo + w_], start=True, stop=True,
                                 tile_position=(0, 64))
                if ch % 2 == 0:
                    nc.scalar.copy(vv_sb[:, 0, lo:lo + w_], pt[0:KSUB, 0:w_])
                    nc.vector.tensor_copy(vv_sb[:, 1, lo:lo + w_],
                                          pt[64:64 + KSUB, 0:w_])
                else:
                    nc.vector.tensor_copy(vv_sb[:, 0, lo:lo + w_],
                                          pt[0:KSUB, 0:w_])
                    nc.scalar.copy(vv_sb[:, 1, lo:lo + w_],
                                   pt[64:64 + KSUB, 0:w_])

            # ---- kT[n, c] = (Wk@xA + bk)^T : NSUB tiles of [128, CI] ----
            for half in range(2):
                pt = avp.tile([128, 512], f32, tag="av")
                js = range(9 * half, 9 * (half + 1))
                for i, j in enumerate(js):
                    nc.tensor.matmul(
                        pt[:, i * CI:(i + 1) * CI],
                        xa_sb[:, j * 128:(j + 1) * 128],
                        wk_sb[:, :],
                        start=True, stop=True,
                    )
                nc.vector.tensor_copy(
                    kT_sb[:, half * 9 * CI:(half + 1) * 9 * CI], pt[:, 0:9 * CI]
                )

            # ---- main loop ----
            e_pairs = [None] * NPAIR
            kts_pairs = [None] * NPAIR
            av_queue = []   # pending spread chunks for the A-group chain
            copy_flip = [0]

            def emit_slot(t):
                at = avp.tile([128, AVS], f32, tag="av")
                for p in range(APAIRS):
                    nc.tensor.matmul(
                        at[0:CI, :],
                        kts_pairs[p][:, :, :],
                        e_pairs[p][:, :, t * AVS:(t + 1) * AVS],
                        start=(p == 0), stop=(p == APAIRS - 1),
                        perf_mode=DR,
                    )
                ot = avo.tile([CI, AVS], f32, tag="avo")
                copy_flip[0] ^= 1
                if copy_flip[0]:
                    nc.scalar.copy(ot[:, :], at[0:CI, :])
                else:
                    nc.vector.tensor_copy(ot[:, :], at[0:CI, :])
                nc.sync.dma_start(av1_d[:, t * AVS:(t + 1) * AVS], ot[:, :])

            sidx = 0
            for j in range(NSUB):
                p, half = divmod(j, 2)
                if half == 0:
                    e_pr = pairs.tile([128, 2, N], fp8, tag="E")
                    kts_pr = stats.tile([128, 2, CI], fp8, tag="kts")
                    e_pairs[p] = e_pr
                    kts_pairs[p] = kts_pr
                e_t = e_pairs[p]
                zp = stats.tile([128, 12], f32, tag="zp")
                for s in range(NSTRIP):
                    sc = scp.tile([128, STRIP], f32, tag="sc")
                    for t3 in range(STRIP // 512):
                        col = s * STRIP + t3 * 512
                        nc.tensor.matmul(
                            sc[:, t3 * 512:(t3 + 1) * 512],
                            vv_sb[:, :, j * 128:(j + 1) * 128],
                            xb_sb[:, :, col:col + 512],
                            start=True, stop=True,
                            perf_mode=DR,
                        )
                    dst = e_t[:, half, s * STRIP:(s + 1) * STRIP]
                    if _assign_act(sidx):
                        nc.scalar.activation(dst, sc[:, :], AF.Exp,
                                             bias=bias[:, :],
                                             accum_out=zp[:, s:s + 1])
                    else:
                        nc.vector._custom_dve(exp8, out=dst, in0=sc[:, :],
                                              s0=C0G, s1=C1G, imm2=0.5,
                                              accum_out=zp[:, s:s + 1])
                    sidx += 1
                    # pace the spread: 18 slots over the ~90 remaining strips
                    remaining = (NSUB - 1 - j) * NSTRIP + (NSTRIP - 1 - s)
                    while av_queue and len(av_queue) * 5 > remaining:
                        emit_slot(av_queue.pop(0))
                z = stats.tile([128, 1], f32, tag="z")
                nc.vector.reduce_sum(z[:, :], zp[:, 0:NSTRIP], axis=AX.X)
                rinv = stats.tile([128, 1], f32, tag="rinv")
                nc.vector.reciprocal(rinv[:, :], z[:, :])
                # x64 keeps the fp8 kts out of subnormal range; phase2's
                # Wfin is pre-divided by 64 to compensate
                nc.vector.tensor_scalar(
                    kts_pairs[p][:, half, :], kT_sb[:, j * CI:(j + 1) * CI],
                    rinv[:, :], 64.0,
                    op0=mybir.AluOpType.mult, op1=mybir.AluOpType.mult,
                )
                if j == 2 * APAIRS - 1:
                    av_queue.extend(range(NAVS))

            # ---- tail: pairs 4-8 chain per chunk, 5 chunks in flight
            # across all 5 PSUM slots; pair-outer order amortizes the
            # stationary loads ----
            for tg in range(0, NAVS, 5):
                ts_ = list(range(tg, min(NAVS, tg + 5)))
                rts = {}
                for i, t in enumerate(ts_):
                    pool, tag = (scp, "sc") if i % 2 == 0 else (avp, "av")
                    rt_tile = pool.tile([128, AVS], f32, tag=tag)
                    rts[t] = rt_tile
                for p in range(APAIRS, NPAIR):
                    for t in ts_:
                        nc.tensor.matmul(
                            rts[t][0:CI, :],
                            kts_pairs[p][:, :, :],
                            e_pairs[p][:, :, t * AVS:(t + 1) * AVS],
                            start=(p == APAIRS), stop=(p == NPAIR - 1),
                            perf_mode=DR,
                        )
                for t in ts_:
                    ot = avo.tile([CI, AVS], f32, tag="avo")
                    if t % 2 == 0:
                        nc.scalar.copy(ot[:, :], rts[t][0:CI, :])
                    else:
                        nc.vector.tensor_copy(ot[:, :], rts[t][0:CI, :])
                    nc.sync.dma_start(av2_d[:, t * AVS:(t + 1) * AVS], ot[:, :])

    nc.compile()
    return nc


def _build_phase2():
    import concourse.bacc as bacc
    import concourse.tile as tile
    from concourse import mybir

    f32 = mybir.dt.float32
    bf16 = mybir.dt.bfloat16
    AF = mybir.ActivationFunctionType
    MQ = N // 4  # 2304 output columns per core

    nc = bacc.Bacc("TRN2", target_bir_lowering=False, debug=False)

    # avs carries [WfinT+cfin | av_sum+ones] side by side; chunked DMAs so
    # the first matmul starts as soon as its slice lands
    avs_d = nc.dram_tensor("avs", [CI + 1, MQ + C], bf16, kind="ExternalInput").ap()
    xbc_d = nc.dram_tensor("xbc", [C, MQ], bf16, kind="ExternalInput").ap()
    out_d = nc.dram_tensor("outc", [C, MQ], f32, kind="ExternalOutput").ap()

    with tile.TileContext(nc) as tc:
        with (
            tc.tile_pool(name="sb", bufs=1) as sb,
            tc.tile_pool(name="tp", bufs=3) as tp,
            tc.tile_pool(name="ps", bufs=6, space="PSUM") as ps,
        ):
            warm = sb.tile([128, 1], f32, tag="warm")
            nc.vector.memset(warm[:, :], 0.0)
            warm2 = sb.tile([128, 1], f32, tag="warm2")
            nc.scalar.activation(warm2[:, :], warm[:, :], AF.Relu)

            av_aug = sb.tile([CI + 1, MQ + C], bf16, tag="avaug")
            nc.sync.dma_start(av_aug[:, 0:C + 512], avs_d[:, 0:C + 512])
            xbc_sb = sb.tile([C, MQ], bf16, tag="xbc")
            nc.gpsimd.dma_start(xbc_sb[:, 0:1024], xbc_d[:, 0:1024])
            for lo in range(C + 512, MQ + C, 1024):
                hi = min(MQ + C, lo + 1024)
                nc.sync.dma_start(av_aug[:, lo:hi], avs_d[:, lo:hi])
            nc.gpsimd.dma_start(xbc_sb[:, 1024:MQ], xbc_d[:, 1024:MQ])
            o_sb = sb.tile([C, MQ], f32, tag="o")

            nstr = (MQ + 511) // 512
            for s in range(nstr):
                sw = min(512, MQ - s * 512)
                sl = slice(s * 512, s * 512 + sw)
                op = ps.tile([128, 512], f32, tag="rp")
                nc.tensor.matmul(
                    op[0:C, 0:sw], av_aug[:, 0:C],
                    av_aug[:, C + s * 512:C + s * 512 + sw],
                    start=True, stop=True,
                )
                t_sb = tp.tile([C, 512], f32, tag="t")
                nc.vector.tensor_tensor(
                    t_sb[:, 0:sw], xbc_sb[:, sl], op[0:C, 0:sw],
                    op=mybir.AluOpType.add,
                )
                nc.scalar.activation(o_sb[:, sl], t_sb[:, 0:sw], AF.Relu)
                nc.sync.dma_start(out_d[:, sl], o_sb[:, sl])

    nc.compile()
    return nc


def _get_programs():
    if "p1" not in _CACHE:
        _CACHE["p1"] = _build_phase1()
        _CACHE["p2"] = _build_phase2()
    return _CACHE["p1"], _CACHE["p2"]


def kernel(xA, xB, Wk, bk, Wv, bv, Wq, bq, Wg,
           g1_gamma, g1_beta, g1_mean, g1_var,
           Wo, bo, g2_gamma, g2_beta, g2_mean, g2_var):
    from concourse.bass_utils import run_bass_kernel_spmd

    p1, p2 = _get_programs()

    xA = np.asarray(xA, np.float32).reshape(B, C, N)
    xB = np.asarray(xB, np.float32).reshape(B, C, N)

    # ---- host-side weight folding (tiny) ----
    s1 = np.asarray(g1_gamma) / np.sqrt(np.asarray(g1_var) + EPS)
    Wg_f = s1[:, None] * np.asarray(Wg)
    c1 = np.asarray(g1_beta) - s1 * np.asarray(g1_mean)
    s2 = np.asarray(g2_gamma) / np.sqrt(np.asarray(g2_var) + EPS)
    Wo_f = s2[:, None] * np.asarray(Wo)
    c2 = s2 * (np.asarray(bo) - np.asarray(g2_mean)) + np.asarray(g2_beta)
    Wfin = (Wo_f @ Wg_f).astype(np.float32)          # [C, CI]
    cfin = (Wo_f @ c1 + c2).astype(np.float32)       # [C]

    # scores fold: S = xA_aug^T M xB_aug,  M = [[Wv^T Wq, Wv^T bq],
    #                                          [bv^T Wq,  bv.bq ]]
    Wv_, bv_ = np.asarray(Wv), np.asarray(bv)
    Wq_, bq_ = np.asarray(Wq), np.asarray(bq)
    M = np.zeros((CAUG, CAUG), np.float32)
    M[:C, :C] = Wv_.T @ Wq_
    M[:C, C] = Wv_.T @ bq_
    M[C, :C] = bv_ @ Wq_
    M[C, C] = bv_ @ bq_
    # stationary halves for vv = M^T xA_aug: mt[:, h*33+e] = M[:, ...]
    Mp = np.zeros((CAUG, 2 * KSUB), np.float32)
    Mp[:, 0:KSUB] = M[:, 0:KSUB]
    Mp[:, KSUB:KSUB + (CAUG - KSUB)] = M[:, KSUB:CAUG]
    mt16 = Mp.astype(BF16)

    wk_aug = np.concatenate([np.asarray(Wk).T, np.asarray(bk)[None, :]], 0).astype(BF16)
    # av_part carries a x64 factor (fp8 subnormal guard) — undo it in Wfin
    wfin_aug = np.concatenate([Wfin.T / 64.0, cfin[None, :]], 0).astype(BF16)

    ones_n = np.ones((1, N), np.float32)

    # xB_aug in the fp8 split layout [33, 2, N] (ch 0-32 | ch 33-64 + pad)
    xb_split = np.zeros((B, KSUB, 2, N), np.float32)
    for b in range(B):
        xb_aug = np.concatenate([xB[b], ones_n], 0)  # [65, N]
        xb_split[b, :, 0, :] = xb_aug[0:KSUB]
        xb_split[b, 0:CAUG - KSUB, 1, :] = xb_aug[KSUB:CAUG]
    xb_split8 = xb_split.reshape(B, KSUB, 2 * N).astype(E4M3)

    # ---- phase 1: per-core (batch, key-row chunk) partial attention ----
    in_maps1 = []
    for core in range(NCORES):
        b, chunk = divmod(core, 4)
        sl = slice(chunk * NCHUNK, (chunk + 1) * NCHUNK)
        in_maps1.append({
            "xb_split": xb_split8[b],
            "xa_aug": np.concatenate([xA[b][:, sl], ones_n[:, sl]], 0).astype(BF16),
            "mt": mt16,
            "wk": wk_aug,
        })
    res1 = run_bass_kernel_spmd(p1, in_maps1, list(range(NCORES)))
    av_parts = [res1.results[i]["av1"] + res1.results[i]["av2"]
                for i in range(NCORES)]

    # ---- phase 2: per-core (batch, query chunk) epilogue ----
    # (the 4-way partial sum happens on host as part of the gather)
    MQ = N // 4
    av_sum = [sum(av_parts[b * 4 + i] for i in range(4)) for b in range(B)]
    ones_mq = np.ones((1, MQ), np.float32)
    in_maps2 = []
    for core in range(NCORES):
        b, mq = divmod(core, 4)
        msl = slice(mq * MQ, (mq + 1) * MQ)
        av_aug = np.concatenate([av_sum[b][:, msl], ones_mq], 0)
        in_maps2.append({
            "avs": np.concatenate([wfin_aug, av_aug], 1).astype(BF16),
            "xbc": np.ascontiguousarray(xB[b][:, msl]).astype(BF16),
        })
    res2 = run_bass_kernel_spmd(p2, in_maps2, list(range(NCORES)))

    out = np.zeros((B, C, N), np.float32)
    for core in range(NCORES):
        b, mq = divmod(core, 4)
        out[b][:, mq * MQ:(mq + 1) * MQ] = res2.results[core]["outc"]
    return out.reshape(B, C, H, W)


# revision 35
# speedup vs baseline: 1.0234x; 1.0063x over previous
"""Trainium2 Bass kernel for nn_MFA_87067577025371.

Architecture (B=2, C=64, Ci=32, H=W=96, N=9216):
  k,v = 1x1conv(xA); q = 1x1conv(xB)
  A   = softmax(v^T q, axis=2)            # [B, N, N], softmax over query dim m
  av  = k @ A                             # [B, Ci, N]
  out = relu(BN2(Wo @ BN1(Wg @ av)) + xB)

Softmax is over the *output* (query) dim of the av contraction, so the row
statistics Z_n = sum_m exp(s[n,m]) only need rows n — we shard over
(batch, key-row chunk): each of 8 cores owns N/4 = 2304 rows of the score
matrix for one batch.

Phase-1 design (per core):
 - scores S = vv^T xB_aug with vv = M^T xA_aug, M = [[Wv^T Wq, Wv^T bq],
   [bv^T Wq, bv.bq]] host-folded: one fp8 DoubleRow matmul per 512-chunk
   (contraction 66 = 2x33 subtiles; xB_aug host-cast into the split fp8
   layout, vv computed on-core and split by two PSUM->SBUF cast copies).
 - exp is split across two engines: ACT runs the true Exp (bias=-c,
   fused row-sum accumulator); DVE runs a custom fused 8-stage op
   ((x*C0+C1)^2+0.5)^8 = (1+t+t^2/2)^8 ~= e^(x-c), t=(x-c)/8, also with
   fused accumulation.  E is stored fp8 (shift c keeps it in range; the
   softmax is shift-invariant so c cancels exactly).
 - av: adjacent row-blocks are paired; each pair is one fp8 DoubleRow
   matmul (2x128 contraction per 0.5-cycle column).  Pairs 0-3 chain in
   PSUM per 512-chunk (slots spread over later blocks' exp strips) and
   stream out as av1; pairs 4-8 chain in the tail (5 chunks in flight
   across the 5 PSUM slots) as av2; the host sums av1+av2 during the
   gather, avoiding any on-core merge serialization.
A tiny second kernel sums the 4 partials per batch and applies the epilogue
(all BN/conv algebra is host-folded into one 64x32 matmul + bias).
"""

import os
import sys

import numpy as np

for _p in ("/opt/trn_rl_repo", "/root/.axon_site/_ro/trn_rl_repo"):
    if os.path.isdir(_p) and _p not in sys.path:
        sys.path.insert(0, _p)

import ml_dtypes  # noqa: E402

BF16 = ml_dtypes.bfloat16
E4M3 = ml_dtypes.float8_e4m3

# ---- problem constants (hardcoded per contract) ----
B, C, CI, H, W = 2, 64, 32, 96, 96
N = H * W                  # 9216
NCORES = 8
NCHUNK = N // 4            # 2304 key rows per core
NSUB = NCHUNK // 128       # 18 blocks of 128 rows
NPAIR = NSUB // 2          # 9 DoubleRow block pairs
STRIP = 1024               # exp strip (2 PSUM banks; 3-deep pipeline)
NSTRIP = N // STRIP        # 9
AVS = 512                  # av matmul chunk
NAVS = N // AVS            # 18
CAUG = C + 1               # 65 (bias row folded in)
KSUB = 33                  # fp8 DoubleRow contraction subtile (2x33 >= 65)
EPS = 1e-5
C_SHIFT = 4.5              # global exponent shift (softmax-invariant)
SQRT2 = 1.4142135623730951

APAIRS = 4                 # pairs 0-3 chain into av_acc (spread)
# pairs 4-8 chain in the tail directly

# exp engine split: a of the 162 strips go to ACT, rest to the DVE custom op.
ACT_STRIPS = 82

_CACHE = {}


def _assign_act(idx, total=NSUB * NSTRIP, a=ACT_STRIPS):
    return ((idx + 1) * a) // total > (idx * a) // total


def _register_exp8():
    """Runtime-register the custom DVE exp-approx op (idempotent)."""
    from concourse import dve_ops
    from concourse.dve_spec import Spec, Src0, C0, C1, C2, sq, AluOp, lower
    from concourse.dve_uop import DveOpSpec

    name = "EXP8_ANT"
    for o in dve_ops.OPS:
        if o.name == name:
            return o
    import numpy as _np

    row = dve_ops._CUSTOM_DVE_ROW_BASE + len(dve_ops.OPS)
    dve_ops._SUB_OPCODE_FOR_NAME[name] = row
    body = sq(sq(sq(sq(Src0 * C0 + C1) + C2)))
    spec = Spec(
        body=body,
        accum=AluOp.ADD,
        reference=lambda in0, in1, s0, s1, imm2: _np.float32(
            ((((in0 * s0 + s1) ** 2 + imm2) ** 2) ** 2) ** 2
        ),
    )
    shas = {
        v: DveOpSpec(name=name, opcode=row, uops=lower(spec, ver=v),
                     rd1_en=False).sha(v)
        for v in ("v3", "v4")
    }
    op = dve_ops.DveOp(name, spec, subdim=False, uops_sha=shas)
    dve_ops.OPS.append(op)
    return op


def _build_phase1():
    import concourse.bacc as bacc
    import concourse.tile as tile
    from concourse import mybir

    f32 = mybir.dt.float32
    bf16 = mybir.dt.bfloat16
    fp8 = mybir.dt.float8e4
    AX = mybir.AxisListType
    AF = mybir.ActivationFunctionType
    ADD = mybir.AluOpType.add
    DR = mybir.MatmulPerfMode.DoubleRow
    exp8 = _register_exp8()
    # DVE approx constants: ((x*C0g + C1g)^2 + 0.5)^8 ~= e^(x - C_SHIFT)
    C0G = 1.0 / (8.0 * SQRT2)
    C1G = (1.0 - C_SHIFT / 8.0) / SQRT2

    nc = bacc.Bacc("TRN2", target_bir_lowering=False, debug=False,
                   dynamic_dma_scratch_size=4096)

    xb_d = nc.dram_tensor("xb_split", [KSUB, 2 * N], fp8, kind="ExternalInput").ap()
    xa_d = nc.dram_tensor("xa_aug", [CAUG, NCHUNK], bf16, kind="ExternalInput").ap()
    mt_d = nc.dram_tensor("mt", [CAUG, 2 * KSUB], bf16, kind="ExternalInput").ap()
    wk_d = nc.dram_tensor("wk", [CAUG, CI], bf16, kind="ExternalInput").ap()
    av1_d = nc.dram_tensor("av1", [CI, N], f32, kind="ExternalOutput").ap()
    av2_d = nc.dram_tensor("av2", [CI, N], f32, kind="ExternalOutput").ap()

    with tile.TileContext(nc) as tc:
        with (
            tc.tile_pool(name="pairs", bufs=NPAIR) as pairs,
            tc.tile_pool(name="pers", bufs=1) as pers,
            tc.tile_pool(name="early", bufs=1) as early,
            tc.tile_pool(name="small", bufs=4) as small,
            tc.tile_pool(name="stats", bufs=10) as stats,
            tc.tile_pool(name="avo", bufs=6) as avo,
            tc.tile_pool(name="scp", bufs=3, space="PSUM") as scp,
            tc.tile_pool(name="avp", bufs=2, space="PSUM") as avp,
        ):
            # ---- warmup: trigger ACT exp-table load before any data ----
            warm = small.tile([128, 1], f32, tag="warm")
            nc.vector.memset(warm[:, :], 0.0)
            warm2 = small.tile([128, 1], f32, tag="warm")
            nc.scalar.activation(warm2[:, :], warm[:, :], AF.Exp)
            bias = small.tile([128, 1], f32, tag="bias")
            nc.vector.memset(bias[:, :], -C_SHIFT)

            # ---- loads: weights + first xB chunks lead; xA on SWDGE ----
            xa_sb = early.tile([CAUG, NCHUNK], bf16, tag="xa")
            nc.gpsimd.dma_start(xa_sb[:], xa_d[:])
            mt_sb = small.tile([CAUG, 2 * KSUB], bf16, tag="mt")
            nc.sync.dma_start(mt_sb[:], mt_d[:])
            wk_sb = small.tile([CAUG, CI], bf16, tag="wk")
            nc.sync.dma_start(wk_sb[:], wk_d[:])
            xb_sb = pers.tile([KSUB, 2, N], fp8, tag="xb")
            xb3 = xb_d[:].rearrange("p (t f) -> p t f", t=2)
            for blk in range(NSTRIP):
                lo, hi = blk * STRIP, (blk + 1) * STRIP
                nc.sync.dma_start(xb_sb[:, :, lo:hi], xb3[:, :, lo:hi])

            vv_sb = pers.tile([KSUB, 2, NCHUNK], fp8, tag="vv")
            kT_sb = pers.tile([128, NSUB * CI], bf16, tag="kT")

            # ---- vv = M^T xa_aug, split into 2x33 fp8 subtiles ----
            # two stationaries land the halves at PSUM partitions 0-32/64-96.
            # chunk ch covers score-stationary blocks 4ch..4ch+3, so only
            # chunk 0 must precede block 0 — the rest slot into block 0's
            # strip cadence to cut the startup stall.
            def emit_vv(ch):
                lo = ch * 512
                w_ = min(512, NCHUNK - lo)
                pt = avp.tile([128, 512], f32, tag="av")
                nc.tensor.matmul(pt[0:KSUB, 0:w_], mt_sb[:, 0:KSUB],
                                 xa_sb[:, lo:lo + w_], start=True, stop=True)
                nc.tensor.matmul(pt[64:64 + KSUB, 0:w_], mt_sb[:, KSUB:2 * KSUB],
                                 xa_sb[:, lo:lo + w_], start=True, stop=True,
                                 tile_position=(0, 64))
                if ch % 2 == 0:
                    nc.scalar.copy(vv_sb[:, 0, lo:lo + w_], pt[0:KSUB, 0:w_])
                    nc.vector.tensor_copy(vv_sb[:, 1, lo:lo + w_],
                                          pt[64:64 + KSUB, 0:w_])
                else:
                    nc.vector.tensor_copy(vv_sb[:, 0, lo:lo + w_],
                                          pt[0:KSUB, 0:w_])
                    nc.scalar.copy(vv_sb[:, 1, lo:lo + w_],
                                   pt[64:64 + KSUB, 0:w_])

            emit_vv(0)
            nvv = (NCHUNK + 511) // 512

            # ---- kT[n, c] = (Wk@xA + bk)^T : NSUB tiles of [128, CI] ----
            for half in range(2):
                pt = avp.tile([128, 512], f32, tag="av")
                js = range(9 * half, 9 * (half + 1))
                for i, j in enumerate(js):
                    nc.tensor.matmul(
                        pt[:, i * CI:(i + 1) * CI],
                        xa_sb[:, j * 128:(j + 1) * 128],
                        wk_sb[:, :],
                        start=True, stop=True,
                    )
                nc.vector.tensor_copy(
                    kT_sb[:, half * 9 * CI:(half + 1) * 9 * CI], pt[:, 0:9 * CI]
                )

            # ---- main loop ----
            e_pairs = [None] * NPAIR
            kts_pairs = [None] * NPAIR
            av_queue = []   # pending spread chunks for the A-group chain
            copy_flip = [0]

            def emit_slot(t):
                at = avp.tile([128, AVS], f32, tag="av")
                for p in range(APAIRS):
                    nc.tensor.matmul(
                        at[0:CI, :],
                        kts_pairs[p][:, :, :],
                        e_pairs[p][:, :, t * AVS:(t + 1) * AVS],
                        start=(p == 0), stop=(p == APAIRS - 1),
                        perf_mode=DR,
                    )
                ot = avo.tile([CI, AVS], f32, tag="avo")
                copy_flip[0] ^= 1
                if copy_flip[0]:
                    nc.scalar.copy(ot[:, :], at[0:CI, :])
                else:
                    nc.vector.tensor_copy(ot[:, :], at[0:CI, :])
                nc.sync.dma_start(av1_d[:, t * AVS:(t + 1) * AVS], ot[:, :])

            sidx = 0
            for j in range(NSUB):
                p, half = divmod(j, 2)
                if half == 0:
                    e_pr = pairs.tile([128, 2, N], fp8, tag="E")
                    kts_pr = stats.tile([128, 2, CI], fp8, tag="kts")
                    e_pairs[p] = e_pr
                    kts_pairs[p] = kts_pr
                e_t = e_pairs[p]
                zp = stats.tile([128, 12], f32, tag="zp")
                for s in range(NSTRIP):
                    sc = scp.tile([128, STRIP], f32, tag="sc")
                    for t3 in range(STRIP // 512):
                        col = s * STRIP + t3 * 512
                        nc.tensor.matmul(
                            sc[:, t3 * 512:(t3 + 1) * 512],
                            vv_sb[:, :, j * 128:(j + 1) * 128],
                            xb_sb[:, :, col:col + 512],
                            start=True, stop=True,
                            perf_mode=DR,
                        )
                    dst = e_t[:, half, s * STRIP:(s + 1) * STRIP]
                    if _assign_act(sidx):
                        nc.scalar.activation(dst, sc[:, :], AF.Exp,
                                             bias=bias[:, :],
                                             accum_out=zp[:, s:s + 1])
                    else:
                        nc.vector._custom_dve(exp8, out=dst, in0=sc[:, :],
                                              s0=C0G, s1=C1G, imm2=0.5,
                                              accum_out=zp[:, s:s + 1])
                    sidx += 1
                    if j == 0 and 1 <= s < nvv:
                        emit_vv(s)
                    # pace the spread: 18 slots over the ~90 remaining strips
                    remaining = (NSUB - 1 - j) * NSTRIP + (NSTRIP - 1 - s)
                    while av_queue and len(av_queue) * 5 > remaining:
                        emit_slot(av_queue.pop(0))
                z = stats.tile([128, 1], f32, tag="z")
                nc.vector.reduce_sum(z[:, :], zp[:, 0:NSTRIP], axis=AX.X)
                rinv = stats.tile([128, 1], f32, tag="rinv")
                nc.vector.reciprocal(rinv[:, :], z[:, :])
                # x64 keeps the fp8 kts out of subnormal range; phase2's
                # Wfin is pre-divided by 64 to compensate
                nc.vector.tensor_scalar(
                    kts_pairs[p][:, half, :], kT_sb[:, j * CI:(j + 1) * CI],
                    rinv[:, :], 64.0,
                    op0=mybir.AluOpType.mult, op1=mybir.AluOpType.mult,
                )
                if j == 2 * APAIRS - 1:
                    av_queue.extend(range(NAVS))

            # ---- tail: pairs 4-8 chain per chunk, 5 chunks in flight
            # across all 5 PSUM slots; pair-outer order amortizes the
            # stationary loads ----
            for tg in range(0, NAVS, 5):
                ts_ = list(range(tg, min(NAVS, tg + 5)))
                rts = {}
                for i, t in enumerate(ts_):
                    pool, tag = (scp, "sc") if i % 2 == 0 else (avp, "av")
                    rt_tile = pool.tile([128, AVS], f32, tag=tag)
                    rts[t] = rt_tile
                for p in range(APAIRS, NPAIR):
                    for t in ts_:
                        nc.tensor.matmul(
                            rts[t][0:CI, :],
                            kts_pairs[p][:, :, :],
                            e_pairs[p][:, :, t * AVS:(t + 1) * AVS],
                            start=(p == APAIRS), stop=(p == NPAIR - 1),
                            perf_mode=DR,
                        )
                for t in ts_:
                    ot = avo.tile([CI, AVS], f32, tag="avo")
                    if t % 2 == 0:
                        nc.scalar.copy(ot[:, :], rts[t][0:CI, :])
                    else:
                        nc.vector.tensor_copy(ot[:, :], rts[t][0:CI, :])
                    nc.sync.dma_start(av2_d[:, t * AVS:(t + 1) * AVS], ot[:, :])

    nc.compile()
    return nc


def _build_phase2():
    import concourse.bacc as bacc
    import concourse.tile as tile
    from concourse import mybir

    f32 = mybir.dt.float32
    bf16 = mybir.dt.bfloat16
    AF = mybir.ActivationFunctionType
    MQ = N // 4  # 2304 output columns per core

    nc = bacc.Bacc("TRN2", target_bir_lowering=False, debug=False)

    # avs carries [WfinT+cfin | av_sum+ones] side by side; chunked DMAs so
    # the first matmul starts as soon as its slice lands
    avs_d = nc.dram_tensor("avs", [CI + 1, MQ + C], bf16, kind="ExternalInput").ap()
    xbc_d = nc.dram_tensor("xbc", [C, MQ], bf16, kind="ExternalInput").ap()
    out_d = nc.dram_tensor("outc", [C, MQ], f32, kind="ExternalOutput").ap()

    with tile.TileContext(nc) as tc:
        with (
            tc.tile_pool(name="sb", bufs=1) as sb,
            tc.tile_pool(name="tp", bufs=3) as tp,
            tc.tile_pool(name="ps", bufs=6, space="PSUM") as ps,
        ):
            warm = sb.tile([128, 1], f32, tag="warm")
            nc.vector.memset(warm[:, :], 0.0)
            warm2 = sb.tile([128, 1], f32, tag="warm2")
            nc.scalar.activation(warm2[:, :], warm[:, :], AF.Relu)

            av_aug = sb.tile([CI + 1, MQ + C], bf16, tag="avaug")
            nc.sync.dma_start(av_aug[:, 0:C + 512], avs_d[:, 0:C + 512])
            xbc_sb = sb.tile([C, MQ], bf16, tag="xbc")
            nc.gpsimd.dma_start(xbc_sb[:, 0:1024], xbc_d[:, 0:1024])
            for lo in range(C + 512, MQ + C, 1024):
                hi = min(MQ + C, lo + 1024)
                nc.sync.dma_start(av_aug[:, lo:hi], avs_d[:, lo:hi])
            nc.gpsimd.dma_start(xbc_sb[:, 1024:MQ], xbc_d[:, 1024:MQ])
            o_sb = sb.tile([C, MQ], f32, tag="o")

            nstr = (MQ + 511) // 512
            for s in range(nstr):
                sw = min(512, MQ - s * 512)
                sl = slice(s * 512, s * 512 + sw)
                op = ps.tile([128, 512], f32, tag="rp")
                nc.tensor.matmul(
                    op[0:C, 0:sw], av_aug[:, 0:C],
                    av_aug[:, C + s * 512:C + s * 512 + sw],
                    start=True, stop=True,
                )
                t_sb = tp.tile([C, 512], f32, tag="t")
                nc.vector.tensor_tensor(
                    t_sb[:, 0:sw], xbc_sb[:, sl], op[0:C, 0:sw],
                    op=mybir.AluOpType.add,
                )
                nc.scalar.activation(o_sb[:, sl], t_sb[:, 0:sw], AF.Relu)
                nc.sync.dma_start(out_d[:, sl], o_sb[:, sl])

    nc.compile()
    return nc


def _get_programs():
    if "p1" not in _CACHE:
        _CACHE["p1"] = _build_phase1()
        _CACHE["p2"] = _build_phase2()
    return _CACHE["p1"], _CACHE["p2"]


def kernel(xA, xB, Wk, bk, Wv, bv, Wq, bq, Wg,
           g1_gamma, g1_beta, g1_mean, g1_var,
           Wo, bo, g2_gamma, g2_beta, g2_mean, g2_var):
    from concourse.bass_utils import run_bass_kernel_spmd

    p1, p2 = _get_programs()

    xA = np.asarray(xA, np.float32).reshape(B, C, N)
    xB = np.asarray(xB, np.float32).reshape(B, C, N)

    # ---- host-side weight folding (tiny) ----
    s1 = np.asarray(g1_gamma) / np.sqrt(np.asarray(g1_var) + EPS)
    Wg_f = s1[:, None] * np.asarray(Wg)
    c1 = np.asarray(g1_beta) - s1 * np.asarray(g1_mean)
    s2 = np.asarray(g2_gamma) / np.sqrt(np.asarray(g2_var) + EPS)
    Wo_f = s2[:, None] * np.asarray(Wo)
    c2 = s2 * (np.asarray(bo) - np.asarray(g2_mean)) + np.asarray(g2_beta)
    Wfin = (Wo_f @ Wg_f).astype(np.float32)          # [C, CI]
    cfin = (Wo_f @ c1 + c2).astype(np.float32)       # [C]

    # scores fold: S = xA_aug^T M xB_aug,  M = [[Wv^T Wq, Wv^T bq],
    #                                          [bv^T Wq,  bv.bq ]]
    Wv_, bv_ = np.asarray(Wv), np.asarray(bv)
    Wq_, bq_ = np.asarray(Wq), np.asarray(bq)
    M = np.zeros((CAUG, CAUG), np.float32)
    M[:C, :C] = Wv_.T @ Wq_
    M[:C, C] = Wv_.T @ bq_
    M[C, :C] = bv_ @ Wq_
    M[C, C] = bv_ @ bq_
    # stationary halves for vv = M^T xA_aug: mt[:, h*33+e] = M[:, ...]
    Mp = np.zeros((CAUG, 2 * KSUB), np.float32)
    Mp[:, 0:KSUB] = M[:, 0:KSUB]
    Mp[:, KSUB:KSUB + (CAUG - KSUB)] = M[:, KSUB:CAUG]
    mt16 = Mp.astype(BF16)

    wk_aug = np.concatenate([np.asarray(Wk).T, np.asarray(bk)[None, :]], 0).astype(BF16)
    # av_part carries a x64 factor (fp8 subnormal guard) — undo it in Wfin
    wfin_aug = np.concatenate([Wfin.T / 64.0, cfin[None, :]], 0).astype(BF16)

    ones_n = np.ones((1, N), np.float32)

    # xB_aug in the fp8 split layout [33, 2, N] (ch 0-32 | ch 33-64 + pad)
    xb_split = np.zeros((B, KSUB, 2, N), np.float32)
    for b in range(B):
        xb_aug = np.concatenate([xB[b], ones_n], 0)  # [65, N]
        xb_split[b, :, 0, :] = xb_aug[0:KSUB]
        xb_split[b, 0:CAUG - KSUB, 1, :] = xb_aug[KSUB:CAUG]
    xb_split8 = xb_split.reshape(B, KSUB, 2 * N).astype(E4M3)

    # ---- phase 1: per-core (batch, key-row chunk) partial attention ----
    in_maps1 = []
    for core in range(NCORES):
        b, chunk = divmod(core, 4)
        sl = slice(chunk * NCHUNK, (chunk + 1) * NCHUNK)
        in_maps1.append({
            "xb_split": xb_split8[b],
            "xa_aug": np.concatenate([xA[b][:, sl], ones_n[:, sl]], 0).astype(BF16),
            "mt": mt16,
            "wk": wk_aug,
        })
    res1 = run_bass_kernel_spmd(p1, in_maps1, list(range(NCORES)))
    av_parts = [res1.results[i]["av1"] + res1.results[i]["av2"]
                for i in range(NCORES)]

    # ---- phase 2: per-core (batch, query chunk) epilogue ----
    # (the 4-way partial sum happens on host as part of the gather)
    MQ = N // 4
    av_sum = [sum(av_parts[b * 4 + i] for i in range(4)) for b in range(B)]
    ones_mq = np.ones((1, MQ), np.float32)
    in_maps2 = []
    for core in range(NCORES):
        b, mq = divmod(core, 4)
        msl = slice(mq * MQ, (mq + 1) * MQ)
        av_aug = np.concatenate([av_sum[b][:, msl], ones_mq], 0)
        in_maps2.append({
            "avs": np.concatenate([wfin_aug, av_aug], 1).astype(BF16),
            "xbc": np.ascontiguousarray(xB[b][:, msl]).astype(BF16),
        })
    res2 = run_bass_kernel_spmd(p2, in_maps2, list(range(NCORES)))

    out = np.zeros((B, C, N), np.float32)
    for core in range(NCORES):
        b, mq = divmod(core, 4)
        out[b][:, mq * MQ:(mq + 1) * MQ] = res2.results[core]["outc"]
    return out.reshape(B, C, H, W)
